# revision 2
# baseline (speedup 1.0000x reference)
"""Deformable KPConv layer on 8 Trainium2 NeuronCores (Bass/Tile), v3.

Data-parallel over the 16384 query points (2048/core), queries in "slot"
order (partition 32*qq+g holds query 4*g+qq).

Distance pipeline per 128-query tile and stage:
  sq[q,k,n] = dot5( S5[q,n,:], C5[q,k,:] )  with
      S5 = (-2*sx, -2*sy, -2*sz, |s|^2, 1)   (host, fp16, packed inner d)
      C5 = (Cx, Cy, Cz, 1, |C|^2)            (fp16; stage-0 from host)
  computed as ONE packed fp16 tensor_tensor mult [P,K,NN,5] (DVE 2x mode)
  + ONE grouped tensor_reduce.  Then clamp>=0, sqrt (ScalarE), min/sub to
  w' = min(d,2)-2 (the -0.5 is folded into conv weights host-side),
  StreamTranspose to edge-slot layout, and four contiguous copies into a
  zero-padded k-major block-diagonal tile wblk3 [slot, (qq,k), g]; the
  neighbor-contraction matmuls stream wblk3[:, :, g] as a strided moving
  operand.  PSUM drains run on ScalarE.
"""

import sys

sys.path.insert(0, "/opt/trn_rl_repo")

import numpy as np
import ml_dtypes

import concourse.bass as bass
import concourse.tile as tile
from concourse import bacc, mybir

N_Q = 16384
N_S = 16384
NN = 32
F_IN = 128
F_OUT = 256
K = 15
DIM = 3
OFF_DIM = DIM * (K - 1)  # 42
N_CORES = 8
P = 128
NG = P // 4  # 32 groups of 4 queries per tile

BF16 = mybir.dt.bfloat16
F16 = mybir.dt.float16
F32 = mybir.dt.float32

AF = mybir.ActivationFunctionType
OP = mybir.AluOpType


def build_nc(qpc: int):
    T = qpc // P

    nc = bacc.Bacc("TRN2", target_bir_lowering=False)

    nfg_d = nc.dram_tensor("nfg", [T, P, NN, F_IN], BF16, kind="ExternalInput")
    s5_d = nc.dram_tensor("s5", [T, P, NN, 5], F16, kind="ExternalInput")
    c50_d = nc.dram_tensor("c50", [T, P, K, 5], F16, kind="ExternalInput")
    c0b_d = nc.dram_tensor("c0b", [T, P, (K - 1) * DIM], F32, kind="ExternalInput")
    dwsb_d = nc.dram_tensor("dwsb", [P, K * OFF_DIM], BF16, kind="ExternalInput")
    wsb_d = nc.dram_tensor("wsb", [P, K * F_OUT], BF16, kind="ExternalInput")
    out_d = nc.dram_tensor("out", [qpc, F_OUT], F32, kind="ExternalOutput")

    with tile.TileContext(nc) as tc:
        with (
            tc.tile_pool(name="const", bufs=1) as cpool,
            tc.tile_pool(name="nf", bufs=4) as nfpool,
            tc.tile_pool(name="s5", bufs=4) as s5pool,
            tc.tile_pool(name="cc", bufs=3) as ccpool,
            tc.tile_pool(name="sq", bufs=3) as sqpool,
            tc.tile_pool(name="wd", bufs=4) as wdpool,
            tc.tile_pool(name="wf", bufs=3) as wfpool,
            tc.tile_pool(name="outp", bufs=2) as opool,
            tc.tile_pool(name="ps", bufs=4, space="PSUM") as pspool,
            tc.tile_pool(name="ps2", bufs=2, space="PSUM") as ps2pool,
        ):
            dwsb = cpool.tile([P, K * OFF_DIM], BF16, tag="dwsb")
            nc.sync.dma_start(out=dwsb[:], in_=dwsb_d[:])
            wsb = cpool.tile([P, K * F_OUT], BF16, tag="wsb")
            nc.sync.dma_start(out=wsb[:], in_=wsb_d[:])
            eps_c = cpool.tile([P, 1], F32, tag="eps")
            nc.vector.memset(eps_c[:], 1e-5)
            zrep = cpool.tile([P, K * NN], F16, tag="zrep")
            nc.vector.memset(zrep[:], 0.0)

            # zero-padded k-major block-diagonal tiles [slot, (qq,k), g]
            wblks = []
            for i in range(8):
                wb = nc.alloc_sbuf_tensor(f"wblk{i}", [P, 4 * K, NG], BF16)
                nc.gpsimd.memset(wb.ap(), 0.0)
                wblks.append(wb)

            for t in range(T):
                nf = nfpool.tile([P, NN, F_IN], BF16, tag="nf")
                nc.sync.dma_start(out=nf[:], in_=nfg_d[t])
                s5 = s5pool.tile([P, NN, 5], F16, tag="s5")
                nc.sync.dma_start(out=s5[:], in_=s5_d[t])
                c50 = ccpool.tile([P, K, 5], F16, tag="c50")
                nc.sync.dma_start(out=c50[:], in_=c50_d[t])
                c0b = ccpool.tile([P, K - 1, DIM], F32, tag="c0b")
                nc.sync.dma_start(
                    out=c0b[:], in_=c0b_d[t].rearrange("p (k d) -> p k d", d=DIM)
                )

                wf_tiles = []
                c5_cur = c50
                for stage in range(2):
                    # sq = dot5(S5, C5): one packed fp16 mult + grouped reduce
                    prod = sqpool.tile([P, K, NN, 5], F16, tag="prod")
                    nc.vector.tensor_tensor(
                        out=prod[:],
                        in0=s5[:].unsqueeze(1).broadcast_to([P, K, NN, 5]),
                        in1=c5_cur[:].unsqueeze(2).broadcast_to([P, K, NN, 5]),
                        op=OP.mult,
                    )
                    sqt = sqpool.tile([P, K, NN], F16, tag="sqt")
                    with nc.allow_low_precision("fp16 5-term dot; validated 9.3e-3"):
                        nc.vector.tensor_reduce(
                            out=sqt[:], in_=prod[:], axis=mybir.AxisListType.X,
                            op=OP.add,
                        )
                    # clamp against fp16 cancellation going negative
                    sqc = sqpool.tile([P, K * NN], F16, tag="sqc")
                    nc.vector.tensor_scalar(
                        out=sqc[:],
                        in0=sqt[:].rearrange("p k n -> p (k n)"),
                        scalar1=0.0,
                        scalar2=None,
                        op0=OP.max,
                    )
                    dts = wdpool.tile([P, K * NN], BF16, tag="dts")
                    nc.scalar.activation(
                        out=dts[:], in_=sqc[:], func=AF.Sqrt, bias=eps_c[:]
                    )
                    # w' = min(d,2)-2 (packed fp16, fast on DVE)
                    wdm = wdpool.tile([P, K * NN], BF16, tag="wdm")
                    nc.vector.tensor_scalar(
                        out=wdm[:], in0=dts[:], scalar1=2.0, scalar2=2.0,
                        op0=OP.min, op1=OP.subtract,
                    )
                    # [q=(qq,g), (k,n)] -> [slot=(qq,n), (k,g)]
                    wtt = wdpool.tile([P, K * NN], BF16, tag="wtt")
                    nc.vector.transpose(out=wtt[:], in_=wdm[:])

                    # block-diagonal: wblk3[32qq+n, 15qq+k, g] = w'
                    # (fp16 -> bf16 casts; contiguous inner g)
                    wblk = wblks[stage * 4 + (t % 4)].ap()
                    wtv = wtt[:].rearrange("p (k g) -> p k g", g=NG)
                    for qq in range(4):
                        dst = wblk[32 * qq : 32 * (qq + 1), K * qq : K * (qq + 1), :]
                        src = wtv[32 * qq : 32 * (qq + 1)]
                        if qq < 2:
                            nc.scalar.copy(out=dst, in_=src)
                        else:
                            nc.gpsimd.tensor_copy(out=dst, in_=src)

                    # neighbor contraction: psum[f, (g8,qq,k)] = nf_g^T . wblk[:, :, g]
                    wf_sb = wfpool.tile([P, K, P], BF16, tag=f"wf{stage}")
                    for b in range(4):
                        psb = pspool.tile([P, 8 * 4 * K], F32, tag="psb")
                        for g8 in range(8):
                            g = b * 8 + g8
                            nc.tensor.matmul(
                                out=psb[:, g8 * 60 : (g8 + 1) * 60],
                                lhsT=nf[:, g, :],
                                rhs=wblk[:, :, g],
                                start=True,
                                stop=True,
                            )
                        drain_src = psb[:].rearrange(
                            "p (gl qq k) -> p k qq gl", gl=8, qq=4
                        )
                        drain_dst = wf_sb[:].rearrange(
                            "p k (qq bb gl) -> p bb k qq gl", qq=4, bb=4, gl=8
                        )[:, b]
                        if b == 3:
                            nc.vector.tensor_copy(out=drain_dst, in_=drain_src)
                        else:
                            nc.scalar.copy(out=drain_dst, in_=drain_src)
                    wf_tiles.append(wf_sb)

                    if stage == 0:
                        psA = ps2pool.tile([P, OFF_DIM], F32, tag="psA")
                        for k in range(K):
                            nc.tensor.matmul(
                                out=psA[:],
                                lhsT=wf_sb[:, k, :],
                                rhs=dwsb[:, k * OFF_DIM : (k + 1) * OFF_DIM],
                                start=(k == 0),
                                stop=(k == K - 1),
                            )
                        # C1 = C0 + offsets (bias folded into c0b host-side)
                        c5t = ccpool.tile([P, K, 5], F16, tag="c5s1")
                        nc.vector.tensor_tensor(
                            out=c5t[:, 1:K, 0:DIM],
                            in0=psA[:].rearrange("p (k d) -> p k d", d=DIM),
                            in1=c0b[:],
                            op=OP.add,
                        )
                        nc.vector.tensor_copy(
                            out=c5t[:, 0, 0:DIM], in_=c50[:, 0, 0:DIM]
                        )
                        nc.vector.memset(c5t[:, :, 3], 1.0)
                        csq = ccpool.tile([P, K, DIM], F32, tag="csq")
                        nc.vector.tensor_tensor(
                            out=csq[:], in0=c5t[:, :, 0:DIM], in1=c5t[:, :, 0:DIM],
                            op=OP.mult,
                        )
                        ccb = ccpool.tile([P, K], F32, tag="ccb")
                        nc.vector.tensor_reduce(
                            out=ccb[:], in_=csq[:], axis=mybir.AxisListType.X,
                            op=OP.add,
                        )
                        nc.vector.tensor_copy(out=c5t[:, :, 4], in_=ccb[:])
                        c5_cur = c5t

                psO = ps2pool.tile([P, F_OUT], F32, tag="psO")
                wf1 = wf_tiles[1]
                for k in range(K):
                    nc.tensor.matmul(
                        out=psO[:],
                        lhsT=wf1[:, k, :],
                        rhs=wsb[:, k * F_OUT : (k + 1) * F_OUT],
                        start=(k == 0),
                        stop=(k == K - 1),
                    )
                out_sb = opool.tile([P, F_OUT], F32, tag="outsb")
                nc.scalar.copy(out=out_sb[:], in_=psO[:])
                nc.sync.dma_start(out=out_d[t * P : (t + 1) * P, :], in_=out_sb[:])

    nc.compile()
    return nc


def _slot_perm():
    s = np.arange(P)
    return 4 * (s % 32) + s // 32


def _prep_shared(support_points, features, K_points, weight, deformable_weight, bias):
    f16 = features.astype(ml_dtypes.bfloat16)
    s5full = np.empty((N_S, 5), dtype=np.float32)
    s5full[:, :3] = -2.0 * support_points
    s5full[:, 3] = (support_points.astype(np.float64) ** 2).sum(1)
    s5full[:, 4] = 1.0
    s5full = s5full.astype(np.float16)
    dwsb = (
        deformable_weight.transpose(1, 0, 2).reshape(F_IN, K * OFF_DIM) * -0.5
    ).astype(ml_dtypes.bfloat16)
    wsb = (
        weight.transpose(1, 0, 2).reshape(F_IN, K * F_OUT) * -0.5
    ).astype(ml_dtypes.bfloat16)
    return f16, s5full, dwsb, wsb


def _prep_core(query_points, neighbors, qpc, f16, s5full, K_points, bias):
    T = qpc // P
    nbr = neighbors.astype(np.int64).reshape(T, P, NN)
    p = np.arange(P)
    g = np.arange(NN)
    ie = nbr[:, (4 * g[None, :] + p[:, None] // 32), (p[:, None] % 32)]
    nfg = np.asarray(f16)[ie]  # [T, P, NN, F_IN] bf16
    qperm = _slot_perm()
    nbr = nbr[:, qperm]
    qp = query_points.reshape(T, P, DIM)[:, qperm]  # [T, P, 3] slot order
    s5 = s5full[nbr]  # [T, P, NN, 5] fp16
    c0 = qp[:, :, None, :] + K_points[None, None, :, :]  # [T, P, K, 3] f32
    c50 = np.zeros((T, P, K, 5), dtype=np.float32)
    c50[..., :3] = c0
    c50[..., 3] = 1.0
    c50[..., 4] = (c0.astype(np.float16).astype(np.float32) ** 2).sum(-1)
    c50 = c50.astype(np.float16)
    c0b = (qp[:, :, None, :] + (K_points[1:] + bias.reshape(K - 1, DIM))[None, None]
           ).reshape(T, P, (K - 1) * DIM).astype(np.float32)
    return nfg, s5, c50, c0b


def prepare(inputs):
    query_points = np.asarray(inputs["query_points"], dtype=np.float32)
    support_points = np.asarray(inputs["support_points"], dtype=np.float32)
    neighbors = np.asarray(inputs["neighbors"])
    features = np.asarray(inputs["features"], dtype=np.float32)
    K_points = np.asarray(inputs["K_points"], dtype=np.float32)
    weight = np.asarray(inputs["weight"], dtype=np.float32)
    deformable_weight = np.asarray(inputs["deformable_weight"], dtype=np.float32)
    bias = np.asarray(inputs["bias"], dtype=np.float32)

    qpc = N_Q // N_CORES
    f16, s5full, dwsb, wsb = _prep_shared(
        support_points, features, K_points, weight, deformable_weight, bias)

    in_maps = []
    for c in range(N_CORES):
        sl = slice(c * qpc, (c + 1) * qpc)
        nfg, s5, c50, c0b = _prep_core(
            query_points[sl], neighbors[sl], qpc, f16, s5full, K_points, bias)
        in_maps.append({
            "nfg": np.asarray(nfg), "s5": s5, "c50": c50, "c0b": c0b,
            "dwsb": np.asarray(dwsb), "wsb": np.asarray(wsb),
        })

    nc = build_nc(qpc)
    return nc, in_maps


def finish(res):
    out = np.concatenate([res.results[c]["out"] for c in range(N_CORES)], axis=0)
    q = np.arange(P)
    islot = 32 * (q % 4) + q // 4
    out = out.reshape(-1, P, F_OUT)[:, islot].reshape(-1, F_OUT)
    return out.astype(np.float32)


def kernel(query_points, support_points, neighbors, features, K_points,
           weight, deformable_weight, bias):
    from concourse.bass_utils import run_bass_kernel_spmd

    nc, in_maps = prepare(dict(
        query_points=query_points, support_points=support_points,
        neighbors=neighbors, features=features, K_points=K_points,
        weight=weight, deformable_weight=deformable_weight, bias=bias))
    res = run_bass_kernel_spmd(nc, in_maps, core_ids=list(range(N_CORES)))
    return finish(res)


# revision 3
# speedup vs baseline: 1.0510x; 1.0510x over previous
"""Deformable KPConv layer on 8 Trainium2 NeuronCores (Bass/Tile), v3.

Data-parallel over the 16384 query points (2048/core), queries in "slot"
order (partition 32*qq+g holds query 4*g+qq).

Distance pipeline per 128-query tile and stage:
  sq[q,k,n] = dot5( S5[q,n,:], C5[q,k,:] )  with
      S5 = (-2*sx, -2*sy, -2*sz, |s|^2, 1)   (host, fp16, packed inner d)
      C5 = (Cx, Cy, Cz, 1, |C|^2)            (fp16; stage-0 from host)
  computed as ONE packed fp16 tensor_tensor mult [P,K,NN,5] (DVE 2x mode)
  + ONE grouped tensor_reduce.  Then clamp>=0, sqrt (ScalarE), min/sub to
  w' = min(d,2)-2 (the -0.5 is folded into conv weights host-side),
  StreamTranspose to edge-slot layout, and four contiguous copies into a
  zero-padded k-major block-diagonal tile wblk3 [slot, (qq,k), g]; the
  neighbor-contraction matmuls stream wblk3[:, :, g] as a strided moving
  operand.  PSUM drains run on ScalarE.
"""

import sys

sys.path.insert(0, "/opt/trn_rl_repo")

import numpy as np
import ml_dtypes

import concourse.bass as bass
import concourse.tile as tile
from concourse import bacc, mybir

N_Q = 16384
N_S = 16384
NN = 32
F_IN = 128
F_OUT = 256
K = 15
DIM = 3
OFF_DIM = DIM * (K - 1)  # 42
N_CORES = 8
P = 128
NG = P // 4  # 32 groups of 4 queries per tile

BF16 = mybir.dt.bfloat16
F16 = mybir.dt.float16
F32 = mybir.dt.float32

AF = mybir.ActivationFunctionType
OP = mybir.AluOpType


def build_nc(qpc: int):
    T = qpc // P

    nc = bacc.Bacc("TRN2", target_bir_lowering=False)

    nfg_d = nc.dram_tensor("nfg", [T, P, NN, F_IN], BF16, kind="ExternalInput")
    s5_d = nc.dram_tensor("s5", [T, P, NN, 5], F16, kind="ExternalInput")
    c50_d = nc.dram_tensor("c50", [T, P, K, 5], F16, kind="ExternalInput")
    c0b_d = nc.dram_tensor("c0b", [T, P, (K - 1) * DIM], F32, kind="ExternalInput")
    dwsb_d = nc.dram_tensor("dwsb", [P, K * OFF_DIM], BF16, kind="ExternalInput")
    wsb_d = nc.dram_tensor("wsb", [P, K * F_OUT], BF16, kind="ExternalInput")
    out_d = nc.dram_tensor("out", [qpc, F_OUT], F32, kind="ExternalOutput")

    with tile.TileContext(nc) as tc:
        with (
            tc.tile_pool(name="const", bufs=1) as cpool,
            tc.tile_pool(name="nf", bufs=4) as nfpool,
            tc.tile_pool(name="s5", bufs=4) as s5pool,
            tc.tile_pool(name="cc", bufs=4) as ccpool,
            tc.tile_pool(name="sq", bufs=3) as sqpool,
            tc.tile_pool(name="wd", bufs=4) as wdpool,
            tc.tile_pool(name="wf", bufs=3) as wfpool,
            tc.tile_pool(name="outp", bufs=2) as opool,
            tc.tile_pool(name="ps", bufs=4, space="PSUM") as pspool,
            tc.tile_pool(name="ps2", bufs=2, space="PSUM") as ps2pool,
        ):
            dwsb = cpool.tile([P, K * OFF_DIM], BF16, tag="dwsb")
            nc.sync.dma_start(out=dwsb[:], in_=dwsb_d[:])
            wsb = cpool.tile([P, K * F_OUT], BF16, tag="wsb")
            nc.sync.dma_start(out=wsb[:], in_=wsb_d[:])
            eps_c = cpool.tile([P, 1], F32, tag="eps")
            nc.vector.memset(eps_c[:], 1e-5)
            zrep = cpool.tile([P, K * NN], F16, tag="zrep")
            nc.vector.memset(zrep[:], 0.0)

            # zero-padded k-major block-diagonal tiles [slot, (qq,k), g]
            wblks = []
            for i in range(8):
                wb = nc.alloc_sbuf_tensor(f"wblk{i}", [P, 4 * K, NG], BF16)
                nc.gpsimd.memset(wb.ap(), 0.0)
                wblks.append(wb)

            def emit_stage(t, stage, st):
                """Emit one stage's chain for tile t using state dict st."""
                c5_cur = st["c50"] if stage == 0 else st["c5t"]
                nf, s5 = st["nf"], st["s5"]
                prod = sqpool.tile([P, K, NN, 5], F16, tag=f"prod{stage}")
                nc.vector.tensor_tensor(
                    out=prod[:],
                    in0=s5[:].unsqueeze(1).broadcast_to([P, K, NN, 5]),
                    in1=c5_cur[:].unsqueeze(2).broadcast_to([P, K, NN, 5]),
                    op=OP.mult,
                )
                sqt = sqpool.tile([P, K, NN], F16, tag=f"sqt{stage}")
                with nc.allow_low_precision("fp16 5-term dot; validated 9.3e-3"):
                    nc.vector.tensor_reduce(
                        out=sqt[:], in_=prod[:], axis=mybir.AxisListType.X,
                        op=OP.add,
                    )
                sqc = sqpool.tile([P, K * NN], F16, tag=f"sqc{stage}")
                nc.vector.tensor_scalar(
                    out=sqc[:],
                    in0=sqt[:].rearrange("p k n -> p (k n)"),
                    scalar1=0.0,
                    scalar2=None,
                    op0=OP.max,
                )
                dts = wdpool.tile([P, K * NN], BF16, tag=f"dts{stage}")
                nc.scalar.activation(
                    out=dts[:], in_=sqc[:], func=AF.Sqrt, bias=eps_c[:]
                )
                wdm = wdpool.tile([P, K * NN], BF16, tag=f"wdm{stage}")
                nc.vector.tensor_scalar(
                    out=wdm[:], in0=dts[:], scalar1=2.0, scalar2=2.0,
                    op0=OP.min, op1=OP.subtract,
                )
                wtt = wdpool.tile([P, K * NN], BF16, tag=f"wtt{stage}")
                nc.vector.transpose(out=wtt[:], in_=wdm[:])

                wblk = wblks[stage * 4 + (t % 4)].ap()
                wtv = wtt[:].rearrange("p (k g) -> p k g", g=NG)
                for qq in range(4):
                    dst = wblk[32 * qq : 32 * (qq + 1), K * qq : K * (qq + 1), :]
                    srcv = wtv[32 * qq : 32 * (qq + 1)]
                    if qq < 2:
                        nc.scalar.copy(out=dst, in_=srcv)
                    else:
                        nc.gpsimd.tensor_copy(out=dst, in_=srcv)

                wf_sb = wfpool.tile([P, K, P], BF16, tag=f"wf{stage}")
                for b in range(4):
                    psb = pspool.tile([P, 8 * 4 * K], F32, tag="psb")
                    for g8 in range(8):
                        g = b * 8 + g8
                        nc.tensor.matmul(
                            out=psb[:, g8 * 60 : (g8 + 1) * 60],
                            lhsT=nf[:, g, :],
                            rhs=wblk[:, :, g],
                            start=True,
                            stop=True,
                        )
                    drain_src = psb[:].rearrange(
                        "p (gl qq k) -> p k qq gl", gl=8, qq=4
                    )
                    drain_dst = wf_sb[:].rearrange(
                        "p k (qq bb gl) -> p bb k qq gl", qq=4, bb=4, gl=8
                    )[:, b]
                    if b == 3:
                        nc.vector.tensor_copy(out=drain_dst, in_=drain_src)
                    else:
                        nc.scalar.copy(out=drain_dst, in_=drain_src)

                if stage == 0:
                    psA = ps2pool.tile([P, OFF_DIM], F32, tag="psA")
                    for k in range(K):
                        nc.tensor.matmul(
                            out=psA[:],
                            lhsT=wf_sb[:, k, :],
                            rhs=dwsb[:, k * OFF_DIM : (k + 1) * OFF_DIM],
                            start=(k == 0),
                            stop=(k == K - 1),
                        )
                    c5t = ccpool.tile([P, K, 5], F16, tag="c5s1")
                    nc.vector.tensor_tensor(
                        out=c5t[:, 1:K, 0:DIM],
                        in0=psA[:].rearrange("p (k d) -> p k d", d=DIM),
                        in1=st["c0b"][:],
                        op=OP.add,
                    )
                    nc.vector.tensor_copy(
                        out=c5t[:, 0, 0:DIM], in_=st["c50"][:, 0, 0:DIM]
                    )
                    nc.vector.memset(c5t[:, :, 3], 1.0)
                    csq = ccpool.tile([P, K, DIM], F32, tag="csq")
                    nc.vector.tensor_tensor(
                        out=csq[:], in0=c5t[:, :, 0:DIM], in1=c5t[:, :, 0:DIM],
                        op=OP.mult,
                    )
                    ccb = ccpool.tile([P, K], F32, tag="ccb")
                    nc.vector.tensor_reduce(
                        out=ccb[:], in_=csq[:], axis=mybir.AxisListType.X,
                        op=OP.add,
                    )
                    nc.vector.tensor_copy(out=c5t[:, :, 4], in_=ccb[:])
                    st["c5t"] = c5t
                else:
                    psO = ps2pool.tile([P, F_OUT], F32, tag="psO")
                    for k in range(K):
                        nc.tensor.matmul(
                            out=psO[:],
                            lhsT=wf_sb[:, k, :],
                            rhs=wsb[:, k * F_OUT : (k + 1) * F_OUT],
                            start=(k == 0),
                            stop=(k == K - 1),
                        )
                    out_sb = opool.tile([P, F_OUT], F32, tag="outsb")
                    nc.scalar.copy(out=out_sb[:], in_=psO[:])
                    nc.sync.dma_start(
                        out=out_d[t * P : (t + 1) * P, :], in_=out_sb[:]
                    )

            # software pipeline: stage-1 of tile t-1 interleaves with
            # stage-0 of tile t so every engine has two independent chains
            prev = None
            for t in range(T):
                st = {"t": t}
                nf = nfpool.tile([P, NN, F_IN], BF16, tag="nf")
                nc.sync.dma_start(out=nf[:], in_=nfg_d[t])
                s5 = s5pool.tile([P, NN, 5], F16, tag="s5")
                nc.sync.dma_start(out=s5[:], in_=s5_d[t])
                c50 = ccpool.tile([P, K, 5], F16, tag="c50")
                nc.sync.dma_start(out=c50[:], in_=c50_d[t])
                c0b = ccpool.tile([P, K - 1, DIM], F32, tag="c0b")
                nc.sync.dma_start(
                    out=c0b[:], in_=c0b_d[t].rearrange("p (k d) -> p k d", d=DIM)
                )
                st.update(nf=nf, s5=s5, c50=c50, c0b=c0b)
                emit_stage(t, 0, st)
                if prev is not None:
                    emit_stage(prev["t"], 1, prev)
                prev = st
            emit_stage(prev["t"], 1, prev)

    nc.compile()
    return nc


def _slot_perm():
    s = np.arange(P)
    return 4 * (s % 32) + s // 32


def _prep_shared(support_points, features, K_points, weight, deformable_weight, bias):
    f16 = features.astype(ml_dtypes.bfloat16)
    s5full = np.empty((N_S, 5), dtype=np.float32)
    s5full[:, :3] = -2.0 * support_points
    s5full[:, 3] = (support_points.astype(np.float64) ** 2).sum(1)
    s5full[:, 4] = 1.0
    s5full = s5full.astype(np.float16)
    dwsb = (
        deformable_weight.transpose(1, 0, 2).reshape(F_IN, K * OFF_DIM) * -0.5
    ).astype(ml_dtypes.bfloat16)
    wsb = (
        weight.transpose(1, 0, 2).reshape(F_IN, K * F_OUT) * -0.5
    ).astype(ml_dtypes.bfloat16)
    return f16, s5full, dwsb, wsb


def _prep_core(query_points, neighbors, qpc, f16, s5full, K_points, bias):
    T = qpc // P
    nbr = neighbors.astype(np.int64).reshape(T, P, NN)
    p = np.arange(P)
    g = np.arange(NN)
    ie = nbr[:, (4 * g[None, :] + p[:, None] // 32), (p[:, None] % 32)]
    nfg = np.asarray(f16)[ie]  # [T, P, NN, F_IN] bf16
    qperm = _slot_perm()
    nbr = nbr[:, qperm]
    qp = query_points.reshape(T, P, DIM)[:, qperm]  # [T, P, 3] slot order
    s5 = s5full[nbr]  # [T, P, NN, 5] fp16
    c0 = qp[:, :, None, :] + K_points[None, None, :, :]  # [T, P, K, 3] f32
    c50 = np.zeros((T, P, K, 5), dtype=np.float32)
    c50[..., :3] = c0
    c50[..., 3] = 1.0
    c50[..., 4] = (c0.astype(np.float16).astype(np.float32) ** 2).sum(-1)
    c50 = c50.astype(np.float16)
    c0b = (qp[:, :, None, :] + (K_points[1:] + bias.reshape(K - 1, DIM))[None, None]
           ).reshape(T, P, (K - 1) * DIM).astype(np.float32)
    return nfg, s5, c50, c0b


def prepare(inputs):
    query_points = np.asarray(inputs["query_points"], dtype=np.float32)
    support_points = np.asarray(inputs["support_points"], dtype=np.float32)
    neighbors = np.asarray(inputs["neighbors"])
    features = np.asarray(inputs["features"], dtype=np.float32)
    K_points = np.asarray(inputs["K_points"], dtype=np.float32)
    weight = np.asarray(inputs["weight"], dtype=np.float32)
    deformable_weight = np.asarray(inputs["deformable_weight"], dtype=np.float32)
    bias = np.asarray(inputs["bias"], dtype=np.float32)

    qpc = N_Q // N_CORES
    f16, s5full, dwsb, wsb = _prep_shared(
        support_points, features, K_points, weight, deformable_weight, bias)

    in_maps = []
    for c in range(N_CORES):
        sl = slice(c * qpc, (c + 1) * qpc)
        nfg, s5, c50, c0b = _prep_core(
            query_points[sl], neighbors[sl], qpc, f16, s5full, K_points, bias)
        in_maps.append({
            "nfg": np.asarray(nfg), "s5": s5, "c50": c50, "c0b": c0b,
            "dwsb": np.asarray(dwsb), "wsb": np.asarray(wsb),
        })

    nc = build_nc(qpc)
    return nc, in_maps


def finish(res):
    out = np.concatenate([res.results[c]["out"] for c in range(N_CORES)], axis=0)
    q = np.arange(P)
    islot = 32 * (q % 4) + q // 4
    out = out.reshape(-1, P, F_OUT)[:, islot].reshape(-1, F_OUT)
    return out.astype(np.float32)


def kernel(query_points, support_points, neighbors, features, K_points,
           weight, deformable_weight, bias):
    from concourse.bass_utils import run_bass_kernel_spmd

    nc, in_maps = prepare(dict(
        query_points=query_points, support_points=support_points,
        neighbors=neighbors, features=features, K_points=K_points,
        weight=weight, deformable_weight=deformable_weight, bias=bias))
    res = run_bass_kernel_spmd(nc, in_maps, core_ids=list(range(N_CORES)))
    return finish(res)
